# revision 13
# baseline (speedup 1.0000x reference)
"""Trainium2 Bass kernel for nn_BasicBlockBi (TBN basic block: 2x ternary-binary conv).

Data-parallel over batch: 4 images per core on 8 NeuronCores.
  - bn+ternarize folds into per-channel thresholds (hi, lo) computed on host
    (block 1) / on device from an AllReduced |bn2(h)| sum (block 2).
  - Ternarize is split across engines and pipelined one image ahead so the
    PE never waits: ci-chunk0 = two Scalar Sign ops + one GpSimd add (gives
    2*q in {-2,0,2}); ci-chunk1 = Vector is_gt + fused is_lt-subtract (gives
    -q in {-1,0,1}).  The mismatched scales/signs are folded into the
    precomputed fp8 weights (chunk0: -0.5*sign(w), chunk1: +1.0*sign(w)) so
    PSUM accumulates -conv_true exactly; the epilogue scalar is -alpha.
  - Each conv runs as 9 shifted-tap DoubleRow fp8 matmuls (K=256) per
    (co-chunk, row-half); per-tap output row/col ranges are restricted so
    out-of-bounds taps contribute nothing (zero-padding without padding).
  - delta2 = 0.7*mean|bn2(h)| over the global batch: per-partition partial
    sums ride along as activation accum_out, are AllReduced across the 8
    cores in-kernel, then broadcast to all partitions with a ones-matmul.
  - A dummy AllReduce at kernel start pays the Q7 collective cold-start;
    bf16 spin matmuls keep the PE HAM clock-gate warm through startup and
    the collective stall.
"""

import os
import sys

for _p in ("/opt/trn_rl_repo", "/root/.axon_site/_ro/trn_rl_repo"):
    if os.path.isdir(_p) and _p not in sys.path:
        sys.path.append(_p)

import numpy as np

import concourse.bacc as bacc
import concourse.tile as tile
from concourse import mybir
from concourse.bass_utils import run_bass_kernel_spmd

B, C, H, W = 32, 256, 32, 32
HW = H * W
NCORES = 8
BL = B // NCORES          # images per core
CCH = C // 128            # channel chunks of 128
EPS = 1e-5
FRAC = 0.7

QDT = mybir.dt.float8e4   # DoubleRow matmul dtype (2x PE throughput); exact for small ints
SDT = mybir.dt.bfloat16   # ternarize intermediate dtype

AOP = mybir.AluOpType
AFT = mybir.ActivationFunctionType
F32 = mybir.dt.float32

# vecs rows
(V_NT1HI, V_NT1LO, V_T1HI, V_T1LO, V_NA1, V_NA2, V_S2, V_B2,
 V_KRS2, V_NKRS2, V_BRS2, V_NBRS2) = range(12)
NVEC = 12

STARTUP_SPINS = 12
STALL_SPINS = 24

TRACE = False
LAST_RESULT = None

_cache: dict = {}


# center tap first: it covers the full tile, so it starts the PSUM group
TAPS = [(1, 1)] + [(kh, kw) for kh in range(3) for kw in range(3) if (kh, kw) != (1, 1)]


def _conv_matmuls(nc, psum_tile, wtiles, qp, co, half):
    """Accumulate the shifted-tap matmuls for one (co chunk, row half).

    fp8 path: q is the unpadded (128, 2, 1024) ternary tile; zero-padding
    semantics come from restricting each tap's output rows/cols to positions
    whose shifted input is in-bounds (other positions get no contribution).
    """
    y0 = 16 * half
    ps_r = psum_tile.rearrange("p (r c) -> p r c", c=32)
    q_r = qp.rearrange("p t (r c) -> p t r c", c=32)
    for idx, (kh, kw) in enumerate(TAPS):
        lo = max(y0, 1 - kh)
        hi = min(y0 + 15, 32 - kh)
        rcnt = hi - lo + 1
        ocol = 1 if kw == 0 else 0
        ccnt = 32 if kw == 1 else 31
        rcol = ocol + kw - 1
        rhs = q_r[:, :, lo + kh - 1 : lo + kh - 1 + rcnt, rcol : rcol + ccnt]
        out_ap = ps_r[:, lo - y0 : lo - y0 + rcnt, ocol : ocol + ccnt]
        nc.tensor.matmul(
            out_ap,
            lhsT=wtiles[kh * 3 + kw][:, :, co * 128 : (co + 1) * 128],
            rhs=rhs,
            start=(idx == 0),
            stop=(idx == 8),
            perf_mode=mybir.MatmulPerfMode.DoubleRow,
            skip_group_check=True,
        )


def _build():
    if "nc" in _cache:
        return _cache["nc"]

    nc = bacc.Bacc("TRN2", num_devices=NCORES)

    x_in = nc.dram_tensor("x", (BL, CCH, 128, HW), F32, kind="ExternalInput")
    # [k, tap, ci, co]: host-pretransposed so the DMA is contiguous per
    # partition; DoubleRow pairs the two ci chunks along dim2
    wshape = (128, 9, CCH, C)
    w1t = nc.dram_tensor("w1t", wshape, QDT, kind="ExternalInput")
    w2t = nc.dram_tensor("w2t", wshape, QDT, kind="ExternalInput")
    vecs = nc.dram_tensor("vecs", (128, NVEC, CCH), F32, kind="ExternalInput")
    out_d = nc.dram_tensor("out", (BL, CCH, 128, HW), F32, kind="ExternalOutput")
    cc_in = nc.dram_tensor("cc_in", (128, 1), F32)
    cc_out = nc.dram_tensor("cc_out", (NCORES, 128, 1), F32, addr_space="Shared")
    cw_in = nc.dram_tensor("cw_in", (128, 1), F32)
    cw_out = nc.dram_tensor("cw_out", (NCORES, 128, 1), F32, addr_space="Shared")

    with tile.TileContext(nc) as tc:
        with (
            tc.tile_pool(name="consts", bufs=1) as consts,
            tc.tile_pool(name="persist", bufs=1) as persist,
            tc.tile_pool(name="tmp", bufs=3) as tmp,
            tc.tile_pool(name="qpool", bufs=3) as qpool,
            tc.tile_pool(name="epi", bufs=4) as epi,
            tc.tile_pool(name="psum", bufs=8, space="PSUM") as psum,
        ):
            # ---- constants + all input DMA kicks, priority order ----
            vtile = consts.tile([128, NVEC, CCH], F32, tag="vecs")
            nc.sync.dma_start(out=vtile, in_=vecs[:])
            vt = {}
            for i in range(NVEC):
                for ci in range(CCH):
                    vt[i, ci] = vtile[:, i, ci : ci + 1]

            xims = {}
            for n in range(BL):
                xim_t = persist.tile([128, CCH, HW], F32, tag=f"x{n}")
                xims[n] = xim_t
            # x0 split into half-image transfers so ternarize can start on
            # rows 0..16 before the full chunk lands; x1-x3 are staggered
            # behind the Sign pairs so early transfers get full bandwidth
            for ci in range(CCH):
                nc.sync.dma_start(out=xims[0][:, ci, 0:544], in_=x_in[0, ci, :, 0:544])
                nc.sync.dma_start(out=xims[0][:, ci, 544:], in_=x_in[0, ci, :, 544:])

            wa = consts.tile([128, 9, CCH, C], QDT, tag="w1all")
            nc.scalar.dma_start(out=wa, in_=w1t[:])
            wb = consts.tile([128, 9, CCH, C], QDT, tag="w2all")
            w1s = {tap: wa[:, tap] for tap in range(9)}
            w2s = {tap: wb[:, tap] for tap in range(9)}

            ones128 = consts.tile([128, 128], F32, tag="ones128")
            nc.vector.memset(ones128[:], 1.0)
            onesb = consts.tile([128, 512], SDT, tag="onesb")
            nc.vector.memset(onesb[:], 1.0)
            onesbl = consts.tile([128, 128], SDT, tag="onesbl")
            nc.vector.memset(onesbl[:], 1.0)
            warm = consts.tile([1, 1], F32, tag="warm")
            nc.scalar.activation(warm, ones128[0:1, 0:1], AFT.Sign, bias=0.0, scale=1.0)

            # spin the PE so the HAM clock-gate is at full rate when the
            # first conv matmul issues
            for _wi in range(STARTUP_SPINS):
                psw = psum.tile([128, 512], F32, tag="ps")
                nc.tensor.matmul(psw, lhsT=onesbl, rhs=onesb, start=True, stop=True)

            partials = consts.tile([128, BL * CCH], F32, tag="partials")

            xt, ht = {}, {}
            for n in range(BL):
                for ci in range(CCH):
                    xt[n, ci] = xims[n][:, ci, :]

            def ternarize(qf, src, nhi0, nlo0, hi1, lo1, add_eng, splits=(HW,)):
                """qf[:,0,:] = sign(s-hi0)+sign(s-lo0); qf[:,1,:] = (s<lo1)-(s>hi1).

                splits: column boundaries; each segment is ternarized
                independently so downstream matmuls can start on the first
                rows before the whole image is done.
                """
                a0 = tmp.tile([128, HW], SDT, tag="t0a")
                b0 = tmp.tile([128, HW], SDT, tag="t0b")
                a1 = tmp.tile([128, HW], SDT, tag="t1a")
                c0 = 0
                for c1 in splits:
                    s = slice(c0, c1)
                    nc.scalar.activation(a0[:, s], src[0][:, s], AFT.Sign, bias=nhi0, scale=1.0)
                    nc.scalar.activation(b0[:, s], src[0][:, s], AFT.Sign, bias=nlo0, scale=1.0)
                    nc.vector.tensor_scalar(
                        out=a1[:, s], in0=src[1][:, s], scalar1=hi1, scalar2=None, op0=AOP.is_gt)
                    nc.vector.scalar_tensor_tensor(
                        out=qf[:, 1, s], in0=src[1][:, s], scalar=lo1, in1=a1[:, s],
                        op0=AOP.is_lt, op1=AOP.subtract)
                    add_eng.tensor_tensor(qf[:, 0, s], a0[:, s], b0[:, s], AOP.add)
                    c0 = c1

            def ternarize_a(qf, n):
                # image 0: Vector adds + half-split for first-matmul latency;
                # later images: adds on GpSimd (emitted ahead of the warmup
                # collective in its FIFO, so launch skew can't block them)
                eng = nc.vector if n == 0 else nc.gpsimd
                splits = (544, HW) if n == 0 else (HW,)
                ternarize(qf, (xt[n, 0], xt[n, 1]),
                          vt[V_NT1HI, 0], vt[V_NT1LO, 0],
                          vt[V_T1HI, 1], vt[V_T1LO, 1], eng, splits)
                # stagger the later x loads behind this image's Sign pair
                # (scalar-queue program order) so early transfers get the full
                # DMA ring bandwidth; w2 rides after x2
                if n + 1 < BL:
                    for ci in range(CCH):
                        nc.scalar.dma_start(
                            out=xims[n + 1][:, ci, :], in_=x_in[n + 1, ci])
                if n == 1:
                    nc.scalar.dma_start(out=wb, in_=w2t[:])

            # ---------- phase A: block 1 + |bn2(h)| partial sums ----------
            qfa = {}
            qf_t = qpool.tile([128, CCH, HW], QDT, tag="qf")
            qfa[0] = qf_t
            ternarize_a(qfa[0], 0)

            for n in range(BL):
                if n + 1 < BL:
                    qf_t = qpool.tile([128, CCH, HW], QDT, tag="qf")
                    qfa[n + 1] = qf_t
                    ternarize_a(qfa[n + 1], n + 1)
                qp = qfa[n]

                for co in range(CCH):
                    htile = persist.tile([128, HW], F32, tag=f"h{n}_{co}")
                    ht[n, co] = htile
                    for half in range(2):
                        ps = psum.tile([128, 512], F32, tag="ps")
                        _conv_matmuls(nc, ps, w1s, qp, co, half)
                        sl = slice(half * 512, (half + 1) * 512)
                        # h = x - a1 * P   (P = -conv_true; one DVE op)
                        nc.vector.scalar_tensor_tensor(
                            out=htile[:, sl],
                            in0=ps,
                            scalar=vt[V_NA1, co],
                            in1=xt[n, co][:, sl],
                            op0=AOP.mult,
                            op1=AOP.add,
                        )
                    # |bn2(h)| with per-partition running sum for delta2
                    zabs = tmp.tile([128, HW], F32, tag="zabs")
                    nc.scalar.activation(
                        out=zabs,
                        in_=htile,
                        func=AFT.Abs,
                        bias=vt[V_B2, co],
                        scale=vt[V_S2, co],
                        accum_out=partials[:, n * CCH + co : n * CCH + co + 1],
                    )

                if n == 1:
                    # dummy AllGather (result discarded), gated on h(1) so it
                    # fires mid-phase-A: pays the Q7/mesh cold-start and
                    # absorbs core launch skew before the real collective.
                    # Firing it earlier makes it FINISH later (the mesh
                    # barrier waits for the slowest core) and FIFO-block the
                    # real collective; firing later exposes the skew.
                    dly = tmp.tile([128, 8], F32, tag="dly")
                    nc.gpsimd.tensor_tensor(
                        dly, ht[1, 1][:, 0:8], ht[1, 1][:, 0:8], AOP.add)
                    nc.gpsimd.collective_compute(
                        "AllGather",
                        AOP.bypass,
                        replica_groups=[list(range(NCORES))],
                        ins=[cw_in[:]],
                        outs=[cw_out[:]],
                    )

            # ---------- delta2 via cross-core AllGather ----------
            ptot = consts.tile([128, 1], F32, tag="ptot")
            nc.vector.tensor_reduce(ptot, partials, axis=mybir.AxisListType.X, op=AOP.add)
            nc.sync.dma_start(out=cc_in[:], in_=ptot)
            nc.gpsimd.collective_compute(
                "AllGather",
                AOP.bypass,
                replica_groups=[list(range(NCORES))],
                ins=[cc_in[:]],
                outs=[cc_out[:]],
            )
            # keep the PE busy (and the HAM gate open) across the collective
            for _wi in range(STALL_SPINS):
                psw = psum.tile([128, 512], F32, tag="ps")
                nc.tensor.matmul(psw, lhsT=onesbl, rhs=onesb, start=True, stop=True)

            red8 = consts.tile([128, NCORES], F32, tag="red8")
            nc.sync.dma_start(out=red8, in_=cc_out[:].rearrange("r p one -> p (r one)"))
            red = consts.tile([128, 1], F32, tag="red")
            nc.vector.tensor_reduce(red, red8, axis=mybir.AxisListType.X, op=AOP.add)
            # broadcast-sum the AllReduced per-partition values to every
            # partition with one ones-matmul (PE is idle here); d2 = total T
            d2bank = psum.tile([128, 512], F32, tag="ps")
            d2 = d2bank[:, 0:1]
            nc.tensor.matmul(d2, lhsT=ones128, rhs=red, start=True, stop=True)
            # phase-B ternarize coefficients, all affine in T
            nt2hi0 = consts.tile([128, 1], F32, tag="nt2hi0")
            nc.vector.scalar_tensor_tensor(
                out=nt2hi0, in0=d2, scalar=vt[V_NKRS2, 0], in1=vt[V_BRS2, 0],
                op0=AOP.mult, op1=AOP.add)
            nt2lo0 = consts.tile([128, 1], F32, tag="nt2lo0")
            nc.vector.scalar_tensor_tensor(
                out=nt2lo0, in0=d2, scalar=vt[V_KRS2, 0], in1=vt[V_BRS2, 0],
                op0=AOP.mult, op1=AOP.add)
            t2hi1 = consts.tile([128, 1], F32, tag="t2hi1")
            nc.vector.scalar_tensor_tensor(
                out=t2hi1, in0=d2, scalar=vt[V_KRS2, 1], in1=vt[V_NBRS2, 1],
                op0=AOP.mult, op1=AOP.add)
            t2lo1 = consts.tile([128, 1], F32, tag="t2lo1")
            nc.vector.scalar_tensor_tensor(
                out=t2lo1, in0=d2, scalar=vt[V_NKRS2, 1], in1=vt[V_NBRS2, 1],
                op0=AOP.mult, op1=AOP.add)

            def ternarize_b(qf, n):
                eng = nc.vector if n == 0 else nc.gpsimd
                splits = (544, HW) if n == 0 else (HW,)
                ternarize(qf, (ht[n, 0], ht[n, 1]), nt2hi0, nt2lo0, t2hi1, t2lo1,
                          eng, splits)

            # ---------- phase B: block 2 ----------
            qfb = {}
            qf_t = qpool.tile([128, CCH, HW], QDT, tag="qfb")
            qfb[0] = qf_t
            ternarize_b(qfb[0], 0)

            for n in range(BL):
                if n + 1 < BL:
                    qf_t = qpool.tile([128, CCH, HW], QDT, tag="qfb")
                    qfb[n + 1] = qf_t
                    ternarize_b(qfb[n + 1], n + 1)
                qp = qfb[n]

                oim = epi.tile([128, CCH, HW], F32, tag="oim")
                for co in range(CCH):
                    for half in range(2):
                        ps = psum.tile([128, 512], F32, tag="ps")
                        _conv_matmuls(nc, ps, w2s, qp, co, half)
                        sl = slice(half * 512, (half + 1) * 512)
                        nc.vector.scalar_tensor_tensor(
                            out=oim[:, co, sl],
                            in0=ps,
                            scalar=vt[V_NA2, co],
                            in1=ht[n, co][:, sl],
                            op0=AOP.mult,
                            op1=AOP.add,
                        )
                        # stream each half out as soon as its epilogue lands
                        nc.scalar.dma_start(
                            out=out_d[n, co, :, sl], in_=oim[:, co, sl]
                        )

    nc.finalize()
    _cache["nc"] = nc
    return nc


def _host_prep(x, w1, w2, gamma1, beta1, mean1, var1, gamma2, beta2, mean2, var2):
    f64 = np.float64
    npq = mybir.dt.np(QDT)

    s1 = (gamma1.astype(f64) / np.sqrt(var1.astype(f64) + EPS))
    b1 = beta1.astype(f64) - mean1.astype(f64) * s1
    assert (s1 > 0).all(), "kernel assumes positive bn scale (gamma>0)"
    # delta1 on host (f64 accumulate)
    z1 = x.astype(f64) * s1[None, :, None, None] + b1[None, :, None, None]
    d1 = FRAC * np.abs(z1).mean()
    t1hi = ((d1 - b1) / s1).astype(np.float32)
    t1lo = ((-d1 - b1) / s1).astype(np.float32)

    s2 = (gamma2.astype(f64) / np.sqrt(var2.astype(f64) + EPS))
    b2 = beta2.astype(f64) - mean2.astype(f64) * s2
    assert (s2 > 0).all(), "kernel assumes positive bn scale (gamma>0)"

    a1 = np.abs(w1.astype(f64)).mean(axis=(1, 2, 3)).astype(np.float32)
    a2 = np.abs(w2.astype(f64)).mean(axis=(1, 2, 3)).astype(np.float32)

    def wsign_t(w):
        # (O, I, 3, 3) -> [tap, ci, k, co]; scale ci-chunk0 by -0.5 (its q is
        # 2*q_true) and chunk1 by -1 is folded as +1 on (-q), so PSUM = -conv
        s = np.sign(w.astype(f64)).transpose(2, 3, 1, 0).reshape(9, CCH, 128, C)
        s = s * np.array([-0.5, 1.0], f64)[None, :, None, None]
        # [tap, ci, k, co] -> [k, tap, ci, co] (contiguous DMA; DoubleRow pairs ci)
        return np.ascontiguousarray(s.transpose(2, 0, 1, 3).astype(npq))

    w1q = wsign_t(w1)
    w2q = wsign_t(w2)

    vecs = np.zeros((NVEC, CCH, 128), np.float32)
    vecs[V_NT1HI] = (-t1hi).reshape(CCH, 128)
    vecs[V_NT1LO] = (-t1lo).reshape(CCH, 128)
    vecs[V_T1HI] = t1hi.reshape(CCH, 128)
    vecs[V_T1LO] = t1lo.reshape(CCH, 128)
    vecs[V_NA1] = (-a1).reshape(CCH, 128)
    vecs[V_NA2] = (-a2).reshape(CCH, 128)
    vecs[V_S2] = s2.astype(np.float32).reshape(CCH, 128)
    vecs[V_B2] = b2.astype(np.float32).reshape(CCH, 128)
    # threshold coefficients, affine in the AllReduced total T:
    #   t2hi = (k/s2) T - b2/s2 ; t2lo = -(k/s2) T - b2/s2 ; k = FRAC/(B*C*H*W)
    k = FRAC / float(B * C * HW)
    vecs[V_KRS2] = (k / s2).astype(np.float32).reshape(CCH, 128)
    vecs[V_NKRS2] = (-k / s2).astype(np.float32).reshape(CCH, 128)
    vecs[V_BRS2] = (b2 / s2).astype(np.float32).reshape(CCH, 128)
    vecs[V_NBRS2] = (-b2 / s2).astype(np.float32).reshape(CCH, 128)
    # [vec, ci, k] -> [k, vec, ci] (contiguous DMA)
    return w1q, w2q, np.ascontiguousarray(vecs.transpose(2, 0, 1))


def make_in_maps(**inputs):
    x = np.ascontiguousarray(inputs["x"], np.float32)
    w1q, w2q, vecs = _host_prep(
        x,
        np.asarray(inputs["w1"], np.float32),
        np.asarray(inputs["w2"], np.float32),
        *[np.asarray(inputs[k], np.float32) for k in (
            "gamma1", "beta1", "mean1", "var1",
            "gamma2", "beta2", "mean2", "var2",
        )],
    )
    in_maps = []
    for i in range(NCORES):
        xs = np.ascontiguousarray(
            x[i * BL : (i + 1) * BL].reshape(BL, CCH, 128, HW)
        )
        in_maps.append({"x": xs, "w1t": w1q, "w2t": w2q, "vecs": vecs})
    return in_maps


def kernel(**inputs) -> np.ndarray:
    global LAST_RESULT
    nc = _build()
    in_maps = make_in_maps(**inputs)
    res = run_bass_kernel_spmd(nc, in_maps, list(range(NCORES)), trace=TRACE)
    LAST_RESULT = res
    out = np.concatenate(
        [res.results[i]["out"].reshape(BL, C, H, W) for i in range(NCORES)], axis=0
    )
    return out.astype(np.float32, copy=False)
